# revision 1
# baseline (speedup 1.0000x reference)
"""Trainium2 Bass kernel: single-head causal attention with RoPE.

Reference computation (per batch b of 4):
  Q = rope(x @ W_Q), K = rope(x @ W_K), V = x @ W_V      x: [4096, 2048], W: [2048, 128]
  out = softmax(mask(Q K^T / sqrt(128))) @ V             out: [4096, 128]

Sharding: 8 cores = 4 batches x 2 sequence-halves. Within a batch, each
128-row query block J is split 64/64 between the two cores (core h owns
rows [128J + 64h, 128J + 64h + 64)). Each core packs its 2048 rows in J
order and processes them as 8 chunks of 256 rows; chunk v (1-based)
attends k-blocks [0, 4v). This gives both cores literally identical
instruction streams (balanced causal work); only input DATA differs.

Layout: projections produce Q^T/K^T with head-dim on partitions (RoPE
pair-permuted via host-permuted W columns so rope pairs become partition
halves). Scores are computed transposed: S^T[k, q] = K-block @ Q^T-chunk;
exp on ACT (no max subtraction -- scores are ~N(0,1), safe in fp32);
causal masking applied multiplicatively after exp (memset dead prefix +
one 128x64 triangle multiply whose content is per-core data); row sums
via ones-matmul; PV accumulates out^T in PSUM; final normalize by
broadcast reciprocal, PE-transpose, DMA out.
"""

import math
import os
import sys

sys.path.insert(0, "/opt/trn_rl_repo")

import numpy as np
import ml_dtypes

import concourse.bass as bass
import concourse.mybir as mybir
import concourse.tile as tile
from concourse import bacc
from concourse.masks import make_identity

BF16 = mybir.dt.bfloat16
F32 = mybir.dt.float32

FULL_CFG = dict(seq=4096, emb=2048, bsz=4)


def build_nc(seq, emb):
    """Build the single-core SPMD program. Same program runs on all cores."""
    NB = seq // 128          # q-blocks per batch
    C = NB // 4              # chunks per core (chunk = 256 rows, 4 groups of 64)
    NE = emb // 128          # emb chunks
    QROWS = seq // 2         # rows owned per core
    RC = 512                 # proj row-chunk (kv)
    NKV = seq // RC          # kv row chunks
    QRC = min(512, QROWS)
    NQC = QROWS // QRC       # q row chunks
    scale = 1.0 / math.sqrt(128.0)

    nc = bacc.Bacc("TRN2")

    xq = nc.declare_dram_parameter("xq", [128, NQC, NE, QRC], BF16, isOutput=False)
    xkv = nc.declare_dram_parameter("xkv", [128, NKV, NE, RC], BF16, isOutput=False)
    wq = nc.declare_dram_parameter("wq", [128, NE, 128], BF16, isOutput=False)
    wk = nc.declare_dram_parameter("wk", [128, NE, 128], BF16, isOutput=False)
    wv = nc.declare_dram_parameter("wv", [128, NE, 128], BF16, isOutput=False)
    sinq = nc.declare_dram_parameter("sinq", [128, QROWS], F32, isOutput=False)
    cosq = nc.declare_dram_parameter("cosq", [128, QROWS], F32, isOutput=False)
    sink = nc.declare_dram_parameter("sink", [128, seq], F32, isOutput=False)
    cosk = nc.declare_dram_parameter("cosk", [128, seq], F32, isOutput=False)
    tri = nc.declare_dram_parameter("tri", [128, 64], BF16, isOutput=False)
    ident = nc.declare_dram_parameter("ident", [128, 128], F32, isOutput=False)
    out = nc.declare_dram_parameter("out", [QROWS, 128], F32, isOutput=True)
    sums_out = nc.declare_dram_parameter("sums", [1, QROWS], F32, isOutput=True)

    with tile.TileContext(nc) as tc:
        const_cm = tc.tile_pool(name="const", bufs=1)
        const = const_cm.__enter__()

        wq_t = const.tile([128, NE, 128], BF16, tag="wq")
        wk_t = const.tile([128, NE, 128], BF16, tag="wk")
        wv_t = const.tile([128, NE, 128], BF16, tag="wv")
        sinq_t = const.tile([128, QROWS], F32, tag="sinq")
        cosq_t = const.tile([128, QROWS], F32, tag="cosq")
        sink_t = const.tile([128, seq], F32, tag="sink")
        cosk_t = const.tile([128, seq], F32, tag="cosk")
        tri_t = const.tile([128, 64], BF16, tag="tri")
        ones_t = const.tile([128, 1], BF16, tag="ones")
        ident_f32 = const.tile([128, 128], F32, tag="idf")
        kt_t = const.tile([128, seq], BF16, tag="kt")       # K'^T roped, global order
        v_t = const.tile([128, NB, 128], BF16, tag="v")     # V [k-block, dh]
        qt_t = const.tile([128, QROWS], BF16, tag="qt")     # Q'^T roped, packed order

        nc.sync.dma_start(out=wq_t[:], in_=wq[:])
        nc.sync.dma_start(out=wk_t[:], in_=wk[:])
        nc.sync.dma_start(out=wv_t[:], in_=wv[:])
        nc.sync.dma_start(out=sinq_t[:], in_=sinq[:])
        nc.sync.dma_start(out=cosq_t[:], in_=cosq[:])
        nc.sync.dma_start(out=sink_t[:], in_=sink[:])
        nc.sync.dma_start(out=cosk_t[:], in_=cosk[:])
        nc.sync.dma_start(out=tri_t[:], in_=tri[:])
        nc.sync.dma_start(out=ident_f32[:], in_=ident[:])
        nc.gpsimd.memset(ones_t[:], 1.0)

        # ---------------- projection phase ----------------
        with tc.tile_pool(name="xs", bufs=2) as xpool, \
             tc.tile_pool(name="ropet", bufs=2) as rpool, \
             tc.tile_pool(name="pps", bufs=2, space="PSUM") as ppool, \
             tc.tile_pool(name="vps", bufs=2, space="PSUM") as vpool:

            def rope_store(ps, sin_sl, cos_sl, dst_sl, n):
                # dst = ps * cosF + swap(ps) * sinS   (per-partition tables)
                swp = rpool.tile([128, n], F32, tag="swp")
                m1 = rpool.tile([128, n], F32, tag="m1")
                nc.scalar.copy(out=swp[0:64, :], in_=ps[64:128, :])
                nc.scalar.copy(out=swp[64:128, :], in_=ps[0:64, :])
                nc.vector.tensor_mul(out=m1[:], in0=ps[:], in1=cos_sl)
                nc.vector.tensor_mul(out=swp[:], in0=swp[:], in1=sin_sl)
                nc.vector.tensor_add(out=dst_sl, in0=m1[:], in1=swp[:])

            for rc in range(NKV):
                xt = xpool.tile([128, NE, RC], BF16, tag="x")
                nc.sync.dma_start(out=xt[:], in_=xkv[:, rc])
                cols = slice(rc * RC, (rc + 1) * RC)
                # K^T proj + rope
                ps = ppool.tile([128, RC], F32, tag="p")
                for e in range(NE):
                    nc.tensor.matmul(ps[:], lhsT=wk_t[:, e], rhs=xt[:, e],
                                     start=(e == 0), stop=(e == NE - 1))
                rope_store(ps, sink_t[:, cols], cosk_t[:, cols], kt_t[:, cols], RC)
                # V proj, direct [rows, dh] orientation (x^T block stationary)
                for s in range(RC // 128):
                    vps = vpool.tile([128, 128], F32, tag="v")
                    for e in range(NE):
                        nc.tensor.matmul(
                            vps[:], lhsT=xt[:, e, s * 128:(s + 1) * 128],
                            rhs=wv_t[:, e],
                            start=(e == 0), stop=(e == NE - 1))
                    nc.scalar.copy(out=v_t[:, rc * (RC // 128) + s], in_=vps[:])

            for rc in range(NQC):
                xt = xpool.tile([128, NE, QRC], BF16, tag="x")
                nc.sync.dma_start(out=xt[:, :, 0:QRC], in_=xq[:, rc])
                cols = slice(rc * QRC, (rc + 1) * QRC)
                ps = ppool.tile([128, QRC], F32, tag="p")
                for e in range(NE):
                    nc.tensor.matmul(ps[:], lhsT=wq_t[:, e], rhs=xt[:, e],
                                     start=(e == 0), stop=(e == NE - 1))
                rope_store(ps, sinq_t[:, cols], cosq_t[:, cols], qt_t[:, cols], QRC)

        # ---------------- attention phase ----------------
        with tc.tile_pool(name="pt", bufs=4) as ptpool, \
             tc.tile_pool(name="fin", bufs=2) as finpool, \
             tc.tile_pool(name="stps", bufs=2, space="PSUM") as stpool, \
             tc.tile_pool(name="pvps", bufs=2, space="PSUM") as pvpool, \
             tc.tile_pool(name="onps", bufs=2, space="PSUM") as onpool, \
             tc.tile_pool(name="tpps", bufs=2, space="PSUM") as tppool:

            for v in range(1, C + 1):
                qsl = qt_t[:, (v - 1) * 256: v * 256]
                kc = 4 * v
                pv_ps = pvpool.tile([128, 256], F32, tag="pv")
                on_ps = onpool.tile([1, 256], F32, tag="on")
                for bb in range(kc):
                    st = stpool.tile([128, 256], F32, tag="st")
                    nc.tensor.matmul(st[:], lhsT=kt_t[:, bb * 128:(bb + 1) * 128],
                                     rhs=qsl, start=True, stop=True)
                    pt = ptpool.tile([128, 256], BF16, tag="pt")
                    nc.scalar.activation(pt[:], st[:],
                                         mybir.ActivationFunctionType.Exp,
                                         scale=scale)
                    d = bb - 4 * (v - 1)
                    if d >= 0:
                        if d > 0:
                            nc.gpsimd.memset(pt[:, 0:64 * d], 0.0)
                        nc.vector.tensor_mul(out=pt[:, 64 * d:64 * d + 64],
                                             in0=pt[:, 64 * d:64 * d + 64],
                                             in1=tri_t[:])
                    nc.tensor.matmul(on_ps[:], lhsT=ones_t[:], rhs=pt[:],
                                     start=(bb == 0), stop=(bb == kc - 1))
                    nc.tensor.matmul(pv_ps[:], lhsT=v_t[:, bb], rhs=pt[:],
                                     start=(bb == 0), stop=(bb == kc - 1))

                # finalize: store row-sums + unnormalized out^T (host divides)
                sums = finpool.tile([1, 256], F32, tag="sums")
                outt = finpool.tile([128, 256], F32, tag="outt")
                nc.scalar.copy(out=sums[:], in_=on_ps[:])
                nc.sync.dma_start(out=sums_out[:, (v - 1) * 256: v * 256],
                                  in_=sums[:])
                nc.scalar.copy(out=outt[:], in_=pv_ps[:])
                for half in range(2):
                    tp = tppool.tile([128, 128], F32, tag="tp")
                    nc.tensor.transpose(tp[:], outt[:, half * 128:(half + 1) * 128],
                                        ident_f32[:])
                    ot = finpool.tile([128, 128], F32, tag="ot")
                    nc.scalar.copy(out=ot[:], in_=tp[:])
                    r0 = (v - 1) * 256 + half * 128
                    nc.sync.dma_start(out=out[r0:r0 + 128, :], in_=ot[:])

        const_cm.__exit__(None, None, None)

    nc.finalize()
    return nc


# ---------------- host-side prep ----------------

def _pack_x_T(xrows, NE, nch, rcs):
    """xrows [rows, emb] f32 -> [128, nch, NE, rcs] bf16 with
    out[p, rc, e, r] = xrows[rc*rcs + r, 128e + p]."""
    rows, emb = xrows.shape
    t = xrows.T.astype(ml_dtypes.bfloat16)          # [emb, rows]
    t = t.reshape(NE, 128, nch, rcs)                 # [e, p, rc, r]
    return np.ascontiguousarray(t.transpose(1, 2, 0, 3))


def _perm_cols(w):
    """Interleaved rope pairs -> half-split: [:,0:64]=even cols, [:,64:]=odd."""
    return np.concatenate([w[:, 0::2], w[:, 1::2]], axis=1)


def _tables(sin_rows, cos_rows):
    """[rows, 64] tables -> sinS^T / cosF^T [128, rows] f32."""
    s = sin_rows.T.astype(np.float32)               # [64, rows]
    c = cos_rows.T.astype(np.float32)
    sinS = np.concatenate([-s, s], axis=0)          # [128, rows]
    cosF = np.concatenate([c, c], axis=0)
    return np.ascontiguousarray(sinS), np.ascontiguousarray(cosF)


def make_in_maps(x, sin, cos, W_Q, W_K, W_V, seq, emb, bsz):
    NB = seq // 128
    NE = emb // 128
    QROWS = seq // 2
    RC = 512
    NKV = seq // RC
    QRC = min(512, QROWS)
    NQC = QROWS // QRC

    wqp = _perm_cols(W_Q)
    wkp = _perm_cols(W_K)

    def wfmt(w):
        return np.ascontiguousarray(
            w.astype(ml_dtypes.bfloat16).reshape(NE, 128, 128).transpose(1, 0, 2))

    wq_h, wk_h, wv_h = wfmt(wqp), wfmt(wkp), wfmt(W_V)
    sink_h, cosk_h = _tables(sin, cos)

    kk = np.arange(128)[:, None]
    qq = np.arange(64)[None, :]
    tri_low = (kk <= qq).astype(ml_dtypes.bfloat16)
    tri_high = (kk <= 64 + qq).astype(ml_dtypes.bfloat16)

    in_maps = []
    rowmaps = []
    for c in range(2 * bsz):
        b, h = c // 2, c % 2
        rows_c = (128 * np.arange(NB)[:, None] + 64 * h + np.arange(64)[None, :]
                  ).reshape(-1)                      # packed J order, 64-row halves
        xb = np.asarray(x[b], dtype=np.float32)
        in_maps.append({
            "xq": _pack_x_T(xb[rows_c], NE, NQC, QRC),
            "xkv": _pack_x_T(xb, NE, NKV, RC),
            "wq": wq_h, "wk": wk_h, "wv": wv_h,
            "sinq": np.ascontiguousarray(sink_h[:, rows_c]),
            "cosq": np.ascontiguousarray(cosk_h[:, rows_c]),
            "sink": sink_h, "cosk": cosk_h,
            "tri": tri_low if h == 0 else tri_high,
            "ident": np.eye(128, dtype=np.float32),
        })
        rowmaps.append((b, rows_c))
    return in_maps, rowmaps


_NC_CACHE = {}


def run(x, sin, cos, W_Q, W_K, W_V, seq, emb, bsz, trace=False):
    from concourse.bass_utils import run_bass_kernel_spmd
    key = (seq, emb)
    if key not in _NC_CACHE:
        _NC_CACHE[key] = build_nc(seq, emb)
    nc = _NC_CACHE[key]
    in_maps, rowmaps = make_in_maps(x, sin, cos, W_Q, W_K, W_V, seq, emb, bsz)
    core_ids = list(range(2 * bsz))
    res = run_bass_kernel_spmd(nc, in_maps, core_ids, trace=trace)
    out_full = np.zeros((bsz, seq, 128), dtype=np.float32)
    for c, (b, rows_c) in enumerate(rowmaps):
        o = np.asarray(res.results[c]["out"])
        s = np.asarray(res.results[c]["sums"]).reshape(-1, 1)
        out_full[b, rows_c, :] = o / s
    return out_full, res


def kernel(x, mask, sin, cos, W_Q, W_V, W_K):
    out, _ = run(x, sin, cos, W_Q, W_K, W_V,
                 FULL_CFG["seq"], FULL_CFG["emb"], FULL_CFG["bsz"])
    return out



# revision 2
# speedup vs baseline: 4.2532x; 4.2532x over previous
"""Trainium2 Bass kernel: single-head causal attention with RoPE.

Reference computation (per batch b of 4):
  Q = rope(x @ W_Q), K = rope(x @ W_K), V = x @ W_V      x: [4096, 2048], W: [2048, 128]
  out = softmax(mask(Q K^T / sqrt(128))) @ V             out: [4096, 128]

The wall-clock cost in this environment is dominated by the host->device
tunnel (~65 MB/s) and single-CPU host packing, not device compute. So this
version ships the *minimum* bytes with *zero* host-side packing passes:

- x is shipped as bf16 obtained by a strided uint16 truncation VIEW of the
  f32 input (no cast pass; the one copy happens inside run_bass_kernel_spmd's
  np.concatenate). Each core receives only its own contiguous half-batch
  (8 MB), the true lower bound for 8 cores.
- Each core computes Q/K (roped) and V for its own 2048 contiguous rows,
  then the two cores of a batch exchange them with a pairwise AllGather
  (on-device DRAM bounce, ~3 MB) so both see the full-batch K/V/Q.
- Query ownership for the attention phase is interleaved (core h owns rows
  128J + 64h + r) which makes causal work and the instruction stream
  identical across cores; the per-core interleaved Q columns are gathered
  on-device with a selection-matrix matmul (sel is per-core DATA).
- All transposes (x -> x^T for the projections, tables, final output) are
  PE transposes on device; rope tables are built on device from small
  per-core bf16 slices; output is normalized on device and shipped back
  as bf16 [2048, 128] per core.
"""

import math
import sys

sys.path.insert(0, "/opt/trn_rl_repo")

import numpy as np
import ml_dtypes

import concourse.bass as bass
import concourse.mybir as mybir
import concourse.tile as tile
from concourse import bacc

BF16 = mybir.dt.bfloat16
F32 = mybir.dt.float32

SEQ, EMB, BSZ, DH = 4096, 2048, 4, 128
HROWS = SEQ // 2          # rows owned per core (contiguous half)
NBLK = HROWS // 128       # 16 own 128-row blocks
NE = EMB // 128           # 16 emb chunks
NB = SEQ // 128           # 32 kv blocks
C = NB // 4               # 8 attention chunks of 256 packed q rows


def build_nc():
    scale = 1.0 / math.sqrt(float(DH))
    nc = bacc.Bacc("TRN2", num_devices=8)

    xh = nc.declare_dram_parameter("xh", [NBLK, 128, EMB], BF16, isOutput=False)
    wq = nc.declare_dram_parameter("wq", [128, NE, 128], BF16, isOutput=False)
    wk = nc.declare_dram_parameter("wk", [128, NE, 128], BF16, isOutput=False)
    wv = nc.declare_dram_parameter("wv", [128, NE, 128], BF16, isOutput=False)
    sinn = nc.declare_dram_parameter("sinn", [128, NBLK, 128], BF16, isOutput=False)
    cosn = nc.declare_dram_parameter("cosn", [128, NBLK, 128], BF16, isOutput=False)
    sel = nc.declare_dram_parameter("sel", [128, 64], BF16, isOutput=False)
    tri = nc.declare_dram_parameter("tri", [128, 64], BF16, isOutput=False)
    out = nc.declare_dram_parameter("out", [HROWS, 128], BF16, isOutput=True)

    ident_bf = nc.inline_tensor(np.eye(128, dtype=ml_dtypes.bfloat16), name="idbf")
    ident_f32 = nc.inline_tensor(np.eye(128, dtype=np.float32), name="idf32")

    # pairwise exchange buffers: sections [q_nat | k_T | v_nat], each [128, 2048]
    ex_in = nc.dram_tensor("ex_in", [128, 3 * HROWS], BF16)
    ex_out = nc.dram_tensor("ex_out", [2, 128, 3 * HROWS], BF16)

    with tile.TileContext(nc) as tc:
        const_cm = tc.tile_pool(name="const", bufs=1)
        cp = const_cm.__enter__()

        wq_t = cp.tile([128, NE, 128], BF16, tag="wq")
        wk_t = cp.tile([128, NE, 128], BF16, tag="wk")
        wv_t = cp.tile([128, NE, 128], BF16, tag="wv")
        sinn_t = cp.tile([128, NBLK, 128], BF16, tag="sinn")
        cosn_t = cp.tile([128, NBLK, 128], BF16, tag="cosn")
        sel_t = cp.tile([128, 64], BF16, tag="sel")
        tri_t = cp.tile([128, 64], BF16, tag="tri")
        idbf_t = cp.tile([128, 128], BF16, tag="idbf")
        idf32_t = cp.tile([128, 128], F32, tag="idf32")
        ones_t = cp.tile([128, 1], BF16, tag="ones")

        sinnF = cp.tile([128, NBLK, 128], F32, tag="sinnF")   # natural f32
        cosnF = cp.tile([128, NBLK, 128], F32, tag="cosnF")
        sinKT = cp.tile([128, HROWS], F32, tag="sinKT")       # K^T orientation f32
        cosKT = cp.tile([128, HROWS], F32, tag="cosKT")

        kt_own = cp.tile([128, HROWS], BF16, tag="kt_own")    # roped K^T, own half
        qn_own = cp.tile([128, HROWS], BF16, tag="qn_own")    # roped Q natural, own half
        vn_own = cp.tile([128, HROWS], BF16, tag="vn_own")    # V natural, own half

        kt_full = cp.tile([128, NB, 128], BF16, tag="kt_full")
        qn_full = cp.tile([128, NB, 128], BF16, tag="qn_full")
        v_full = cp.tile([128, NB, 128], BF16, tag="v_full")
        qt = cp.tile([128, HROWS], BF16, tag="qt")            # gathered Q^T, packed

        nc.sync.dma_start(out=wq_t[:], in_=wq[:])
        nc.sync.dma_start(out=wk_t[:], in_=wk[:])
        nc.sync.dma_start(out=wv_t[:], in_=wv[:])
        nc.sync.dma_start(out=sinn_t[:], in_=sinn[:])
        nc.sync.dma_start(out=cosn_t[:], in_=cosn[:])
        nc.sync.dma_start(out=sel_t[:], in_=sel[:])
        nc.sync.dma_start(out=tri_t[:], in_=tri[:])
        nc.sync.dma_start(out=idbf_t[:], in_=ident_bf[:])
        nc.sync.dma_start(out=idf32_t[:], in_=ident_f32[:])
        nc.gpsimd.memset(ones_t[:], 1.0)

        # ---------------- phase 1: tables + projections (own half) ----------
        with tc.tile_pool(name="xn", bufs=2) as xnpool, \
             tc.tile_pool(name="xT", bufs=2) as xTpool, \
             tc.tile_pool(name="rp", bufs=2) as rpool, \
             tc.tile_pool(name="tps", bufs=2, space="PSUM") as tppool, \
             tc.tile_pool(name="kps", bufs=2, space="PSUM") as kpool, \
             tc.tile_pool(name="vqs", bufs=2, space="PSUM") as vqpool:

            # f32 natural tables
            nc.scalar.copy(out=sinnF[:], in_=sinn_t[:])
            nc.scalar.copy(out=cosnF[:], in_=cosn_t[:])
            # K^T-orientation tables by PE-transposing natural blocks
            for jg in range(NBLK):
                tp1 = tppool.tile([128, 128], BF16, tag="tp")
                nc.tensor.transpose(tp1[:], sinn_t[:, jg], idbf_t[:])
                nc.scalar.copy(out=sinKT[:, jg * 128:(jg + 1) * 128], in_=tp1[:])
                tp2 = tppool.tile([128, 128], BF16, tag="tp")
                nc.tensor.transpose(tp2[:], cosn_t[:, jg], idbf_t[:])
                nc.scalar.copy(out=cosKT[:, jg * 128:(jg + 1) * 128], in_=tp2[:])

            def rope_kt(ps, cols):
                """K^T-orientation rope: partition-structured tables."""
                swp = rpool.tile([128, 512], F32, tag="swp")
                m1 = rpool.tile([128, 512], F32, tag="m1")
                nc.scalar.copy(out=swp[0:64, :], in_=ps[64:128, :])
                nc.scalar.copy(out=swp[64:128, :], in_=ps[0:64, :])
                nc.vector.tensor_mul(out=m1[:], in0=ps[:], in1=cosKT[:, cols])
                nc.vector.tensor_mul(out=swp[:], in0=swp[:], in1=sinKT[:, cols])
                nc.vector.tensor_add(out=kt_own[:, cols], in0=m1[:], in1=swp[:])

            def rope_nat(ps, jg):
                """natural-orientation rope: free-structured tables."""
                swp = rpool.tile([128, 128], F32, tag="swn")
                m1 = rpool.tile([128, 128], F32, tag="mn")
                nc.scalar.copy(out=swp[:, 0:64], in_=ps[:, 64:128])
                nc.scalar.copy(out=swp[:, 64:128], in_=ps[:, 0:64])
                nc.vector.tensor_mul(out=m1[:], in0=ps[:], in1=cosnF[:, jg])
                nc.vector.tensor_mul(out=swp[:], in0=swp[:], in1=sinnF[:, jg])
                nc.vector.tensor_add(out=qn_own[:, jg * 128:(jg + 1) * 128],
                                     in0=m1[:], in1=swp[:])

            for rc in range(NBLK // 4):
                xT = xTpool.tile([128, NE, 512], BF16, tag="xT")
                for j in range(4):
                    xn = xnpool.tile([128, EMB], BF16, tag="xn")
                    nc.sync.dma_start(out=xn[:], in_=xh[4 * rc + j])
                    for e in range(NE):
                        tp = tppool.tile([128, 128], BF16, tag="tp")
                        nc.tensor.transpose(tp[:], xn[:, e * 128:(e + 1) * 128],
                                            idbf_t[:])
                        nc.scalar.copy(out=xT[:, e, j * 128:(j + 1) * 128],
                                       in_=tp[:])
                # K^T projection + rope (512 cols at once)
                kps = kpool.tile([128, 512], F32, tag="kps")
                for e in range(NE):
                    nc.tensor.matmul(kps[:], lhsT=wk_t[:, e], rhs=xT[:, e],
                                     start=(e == 0), stop=(e == NE - 1))
                rope_kt(kps, slice(rc * 512, (rc + 1) * 512))
                # V and Q natural per 128-row block
                for j in range(4):
                    jg = 4 * rc + j
                    bsl = slice(j * 128, (j + 1) * 128)
                    vps = vqpool.tile([128, 128], F32, tag="vps")
                    for e in range(NE):
                        nc.tensor.matmul(vps[:], lhsT=xT[:, e, bsl],
                                         rhs=wv_t[:, e],
                                         start=(e == 0), stop=(e == NE - 1))
                    nc.scalar.copy(out=vn_own[:, jg * 128:(jg + 1) * 128],
                                   in_=vps[:])
                    qps = vqpool.tile([128, 128], F32, tag="qps")
                    for e in range(NE):
                        nc.tensor.matmul(qps[:], lhsT=xT[:, e, bsl],
                                         rhs=wq_t[:, e],
                                         start=(e == 0), stop=(e == NE - 1))
                    rope_nat(qps, jg)

        # ---------------- phase 2: pairwise exchange ------------------------
        nc.sync.dma_start(out=ex_in[:, 0:HROWS], in_=qn_own[:])
        nc.sync.dma_start(out=ex_in[:, HROWS:2 * HROWS], in_=kt_own[:])
        nc.sync.dma_start(out=ex_in[:, 2 * HROWS:3 * HROWS], in_=vn_own[:])
        nc.gpsimd.collective_compute(
            "AllGather",
            mybir.AluOpType.bypass,
            replica_groups=[[0, 1], [2, 3], [4, 5], [6, 7]],
            ins=[ex_in[:]],
            outs=[ex_out[:]],
        )
        for g in range(2):
            hb = slice(g * NBLK, (g + 1) * NBLK)
            nc.sync.dma_start(out=qn_full[:, hb], in_=ex_out[g, :, 0:HROWS])
            nc.sync.dma_start(out=kt_full[:, hb],
                              in_=ex_out[g, :, HROWS:2 * HROWS])
            nc.sync.dma_start(out=v_full[:, hb],
                              in_=ex_out[g, :, 2 * HROWS:3 * HROWS])

        # ---------------- phase 3: gather interleaved Q^T -------------------
        with tc.tile_pool(name="gps", bufs=2, space="PSUM") as gpool:
            for J in range(NB):
                gps = gpool.tile([128, 64], F32, tag="g")
                nc.tensor.matmul(gps[:], lhsT=qn_full[:, J], rhs=sel_t[:],
                                 start=True, stop=True)
                nc.scalar.copy(out=qt[:, J * 64:(J + 1) * 64], in_=gps[:])

        # ---------------- phase 4: attention --------------------------------
        with tc.tile_pool(name="pt", bufs=4) as ptpool, \
             tc.tile_pool(name="fin", bufs=2) as finpool, \
             tc.tile_pool(name="stps", bufs=2, space="PSUM") as stpool, \
             tc.tile_pool(name="pvps", bufs=1, space="PSUM") as pvpool, \
             tc.tile_pool(name="sps", bufs=1, space="PSUM") as spool, \
             tc.tile_pool(name="tpps", bufs=1, space="PSUM") as tppool2:

            for v in range(1, C + 1):
                qsl = qt[:, (v - 1) * 256: v * 256]
                kc = 4 * v
                pv_ps = pvpool.tile([128, 256], F32, tag="pv")
                sa_ps = spool.tile([128, 1], F32, tag="sa")
                sb_ps = spool.tile([128, 1], F32, tag="sb")
                for bb in range(kc):
                    st = stpool.tile([128, 256], F32, tag="st")
                    nc.tensor.matmul(st[:], lhsT=kt_full[:, bb], rhs=qsl,
                                     start=True, stop=True)
                    pt = ptpool.tile([128, 256], BF16, tag="pt")
                    nc.scalar.activation(pt[:], st[:],
                                         mybir.ActivationFunctionType.Exp,
                                         scale=scale)
                    d = bb - 4 * (v - 1)
                    if d >= 0:
                        if d > 0:
                            nc.gpsimd.memset(pt[:, 0:64 * d], 0.0)
                        nc.vector.tensor_mul(out=pt[:, 64 * d:64 * d + 64],
                                             in0=pt[:, 64 * d:64 * d + 64],
                                             in1=tri_t[:])
                    nc.tensor.matmul(sa_ps[:], lhsT=pt[:, 0:128], rhs=ones_t[:],
                                     start=(bb == 0), stop=(bb == kc - 1))
                    nc.tensor.matmul(sb_ps[:], lhsT=pt[:, 128:256], rhs=ones_t[:],
                                     start=(bb == 0), stop=(bb == kc - 1))
                    nc.tensor.matmul(pv_ps[:], lhsT=v_full[:, bb], rhs=pt[:],
                                     start=(bb == 0), stop=(bb == kc - 1))

                # finalize: transpose out^T back to natural, divide by sums
                outt = finpool.tile([128, 256], F32, tag="outt")
                nc.scalar.copy(out=outt[:], in_=pv_ps[:])
                srec = finpool.tile([128, 2], F32, tag="srec")
                nc.vector.reciprocal(out=srec[:, 0:1], in_=sa_ps[:])
                nc.vector.reciprocal(out=srec[:, 1:2], in_=sb_ps[:])
                for half in range(2):
                    tp = tppool2.tile([128, 128], F32, tag="tp")
                    nc.tensor.transpose(tp[:], outt[:, half * 128:(half + 1) * 128],
                                        idf32_t[:])
                    ot = finpool.tile([128, 128], BF16, tag="ot")
                    nc.vector.tensor_scalar_mul(out=ot[:], in0=tp[:],
                                                scalar1=srec[:, half:half + 1])
                    r0 = (v - 1) * 256 + half * 128
                    nc.sync.dma_start(out=out[r0:r0 + 128, :], in_=ot[:])

        const_cm.__exit__(None, None, None)

    nc.finalize()
    return nc


# ---------------- host-side prep ----------------

def _bf16_trunc_view(a_f32):
    """f32 ndarray -> bf16 truncation as a zero-copy strided view."""
    v = a_f32.view(np.uint16)[..., 1::2]
    return v.view(ml_dtypes.bfloat16)


def _bf16_to_f32(a_bf16):
    """fast widening cast (ml_dtypes' own astype is slow on this host)."""
    u = np.asarray(a_bf16).view(np.uint16).astype(np.uint32) << 16
    return u.view(np.float32)


def _perm_cols(w):
    """Interleaved rope pairs -> half-split: [:,0:64]=even cols, [:,64:]=odd."""
    return np.concatenate([w[:, 0::2], w[:, 1::2]], axis=1)


def _wfmt(w):
    return np.ascontiguousarray(
        w.astype(ml_dtypes.bfloat16).reshape(NE, 128, 128).transpose(1, 0, 2))


def _tblfmt(t_half):
    """[2048, 128] table slice -> [128, 16, 128] partition-first bf16."""
    return np.ascontiguousarray(
        t_half.reshape(NBLK, 128, 128).transpose(1, 0, 2))


def make_in_maps(x, sin, cos, W_Q, W_K, W_V):
    xb = _bf16_trunc_view(np.ascontiguousarray(x) if not x.flags.c_contiguous else x)

    wq_h = _wfmt(_perm_cols(W_Q))
    wk_h = _wfmt(_perm_cols(W_K))
    wv_h = _wfmt(W_V)

    # natural-orientation rope tables [4096, 128]: [-sin|sin], [cos|cos]
    sinS = np.concatenate([-sin, sin], axis=1).astype(ml_dtypes.bfloat16)
    cosF = np.concatenate([cos, cos], axis=1).astype(ml_dtypes.bfloat16)
    tbl = {}
    for h in range(2):
        rows = slice(HROWS * h, HROWS * (h + 1))
        tbl[h] = (_tblfmt(sinS[rows]), _tblfmt(cosF[rows]))

    eye = np.eye(128, dtype=ml_dtypes.bfloat16)
    sel = {h: np.ascontiguousarray(eye[:, 64 * h:64 * h + 64]) for h in range(2)}
    kk = np.arange(128)[:, None]
    qq = np.arange(64)[None, :]
    tri = {0: (kk <= qq).astype(ml_dtypes.bfloat16),
           1: (kk <= 64 + qq).astype(ml_dtypes.bfloat16)}

    in_maps = []
    for c in range(2 * BSZ):
        b, h = c // 2, c % 2
        in_maps.append({
            "xh": xb[b, HROWS * h:HROWS * (h + 1)].reshape(NBLK, 128, EMB),
            "wq": wq_h, "wk": wk_h, "wv": wv_h,
            "sinn": tbl[h][0], "cosn": tbl[h][1],
            "sel": sel[h], "tri": tri[h],
        })
    return in_maps


_NC_CACHE = {}


def run(x, sin, cos, W_Q, W_K, W_V, trace=False):
    from concourse.bass_utils import run_bass_kernel_spmd
    if "nc" not in _NC_CACHE:
        _NC_CACHE["nc"] = build_nc()
    nc = _NC_CACHE["nc"]
    in_maps = make_in_maps(x, sin, cos, W_Q, W_K, W_V)
    res = run_bass_kernel_spmd(nc, in_maps, list(range(2 * BSZ)), trace=trace)
    out_full = np.empty((BSZ, SEQ, 128), dtype=np.float32)
    ov = out_full.reshape(BSZ, NB, 2, 64, 128)
    for c in range(2 * BSZ):
        b, h = c // 2, c % 2
        o = _bf16_to_f32(res.results[c]["out"]).reshape(NB, 64, 128)
        ov[b, :, h] = o
    return out_full, res


def kernel(x, mask, sin, cos, W_Q, W_V, W_K):
    out, _ = run(np.asarray(x), np.asarray(sin), np.asarray(cos),
                 np.asarray(W_Q), np.asarray(W_K), np.asarray(W_V))
    return out


# revision 15
# speedup vs baseline: 4.2567x; 1.0008x over previous
"""Trainium2 Bass kernel: single-head causal attention with RoPE.

Reference computation (per batch b of 4):
  Q = rope(x @ W_Q), K = rope(x @ W_K), V = x @ W_V      x: [4096, 2048], W: [2048, 128]
  out = softmax(mask(Q K^T / sqrt(128))) @ V             out: [4096, 128]

The wall-clock cost in this environment is dominated by the host->device
tunnel (~65 MB/s) and single-CPU host packing, not device compute. So this
version ships the *minimum* bytes with *zero* host-side packing passes:

- x is shipped as bf16 obtained by a strided uint16 truncation VIEW of the
  f32 input (no cast pass; the one copy happens inside run_bass_kernel_spmd's
  np.concatenate). Each core receives only its own contiguous half-batch
  (8 MB), the true lower bound for 8 cores.
- Each core computes Q/K (roped) and V for its own 2048 contiguous rows,
  then the two cores of a batch exchange them with a pairwise AllGather
  (on-device DRAM bounce, ~3 MB) so both see the full-batch K/V/Q.
- Query ownership for the attention phase is interleaved (core h owns rows
  128J + 64h + r) which makes causal work and the instruction stream
  identical across cores; the per-core interleaved Q columns are gathered
  on-device with a selection-matrix matmul (sel is per-core DATA).
- All transposes (x -> x^T for the projections, tables, final output) are
  PE transposes on device; rope tables are built on device from small
  per-core bf16 slices; output is normalized on device and shipped back
  as bf16 [2048, 128] per core.
"""

import math
import sys

sys.path.insert(0, "/opt/trn_rl_repo")

import numpy as np
import ml_dtypes

import concourse.bass as bass
import concourse.mybir as mybir
import concourse.tile as tile
from concourse import bacc

BF16 = mybir.dt.bfloat16
F16 = mybir.dt.float16
F32 = mybir.dt.float32

SEQ, EMB, BSZ, DH = 4096, 2048, 4, 128
HROWS = SEQ // 2          # rows owned per core (contiguous half)
NBLK = HROWS // 128       # 16 own 128-row blocks
NE = EMB // 128           # 16 emb chunks
NB = SEQ // 128           # 32 kv blocks
C = NB // 4               # 8 attention chunks of 256 packed q rows


def build_nc():
    scale = 1.0 / math.sqrt(float(DH))
    nc = bacc.Bacc("TRN2", num_devices=8)

    xh = nc.declare_dram_parameter("xh", [NBLK, 128, EMB], BF16, isOutput=False)
    # weights sharded 8 ways over cores: stacked [wq|wk|wv] = 48 blocks, 6/core
    wsh = nc.declare_dram_parameter("wsh", [128, 6, 128], BF16, isOutput=False)
    sinr = nc.declare_dram_parameter("sinr", [128, NBLK, 64], F16, isOutput=False)
    cosr = nc.declare_dram_parameter("cosr", [128, NBLK, 64], F16, isOutput=False)
    sel = nc.declare_dram_parameter("sel", [128, 64], BF16, isOutput=False)
    tri = nc.declare_dram_parameter("tri", [128, 64], BF16, isOutput=False)
    out = nc.declare_dram_parameter("out", [HROWS, 128], BF16, isOutput=True)

    ident_bf = nc.inline_tensor(np.eye(128, dtype=ml_dtypes.bfloat16), name="idbf")
    ident_f32 = nc.inline_tensor(np.eye(128, dtype=np.float32), name="idf32")

    # pairwise exchange buffers: sections [q_nat | k_T | v_nat], each [128, 2048]
    ex_in = nc.dram_tensor("ex_in", [128, 3 * HROWS], BF16)
    ex_out = nc.dram_tensor("ex_out", [2, 128, 3 * HROWS], BF16)
    # weight-reassembly AllGather (collectives cannot read IO tensors directly)
    wag_in = nc.dram_tensor("wag_in", [128, 6, 128], BF16)
    wag_out = nc.dram_tensor("wag_out", [8, 128, 6, 128], BF16)

    with tile.TileContext(nc) as tc:
        const_cm = tc.tile_pool(name="const", bufs=1)
        cp = const_cm.__enter__()

        w_all = cp.tile([128, 48, 128], BF16, tag="w_all")
        wqb = lambda e: w_all[:, e]
        wkb = lambda e: w_all[:, NE + e]
        wvb = lambda e: w_all[:, 2 * NE + e]
        sinr_t = cp.tile([128, NBLK, 64], F16, tag="sinr")
        cosr_t = cp.tile([128, NBLK, 64], F16, tag="cosr")
        sel_t = cp.tile([128, 64], BF16, tag="sel")
        tri_t = cp.tile([128, 64], BF16, tag="tri")
        idbf_t = cp.tile([128, 128], BF16, tag="idbf")
        idf32_t = cp.tile([128, 128], F32, tag="idf32")
        ones_t = cp.tile([128, 1], BF16, tag="ones")

        sinnF = cp.tile([128, NBLK, 128], F32, tag="sinnF")   # natural f32
        cosnF = cp.tile([128, NBLK, 128], F32, tag="cosnF")
        sinKT = cp.tile([128, HROWS], F32, tag="sinKT")       # K^T orientation f32
        cosKT = cp.tile([128, HROWS], F32, tag="cosKT")

        kt_own = cp.tile([128, HROWS], BF16, tag="kt_own")    # roped K^T, own half
        qn_own = cp.tile([128, HROWS], BF16, tag="qn_own")    # roped Q natural, own half
        vn_own = cp.tile([128, HROWS], BF16, tag="vn_own")    # V natural, own half

        kt_full = cp.tile([128, NB, 128], BF16, tag="kt_full")
        qn_full = cp.tile([128, NB, 128], BF16, tag="qn_full")
        v_full = cp.tile([128, NB, 128], BF16, tag="v_full")
        qt = cp.tile([128, HROWS], BF16, tag="qt")            # gathered Q^T, packed

        nc.sync.dma_start(out=sinr_t[:], in_=sinr[:])
        nc.sync.dma_start(out=cosr_t[:], in_=cosr[:])
        nc.sync.dma_start(out=sel_t[:], in_=sel[:])
        nc.sync.dma_start(out=tri_t[:], in_=tri[:])
        nc.sync.dma_start(out=idbf_t[:], in_=ident_bf[:])
        nc.sync.dma_start(out=idf32_t[:], in_=ident_f32[:])
        nc.gpsimd.memset(ones_t[:], 1.0)

        # reassemble full weights from the 8 per-core shards
        wsh_t = cp.tile([128, 6, 128], BF16, tag="wsh")
        nc.sync.dma_start(out=wsh_t[:], in_=wsh[:])
        nc.sync.dma_start(out=wag_in[:], in_=wsh_t[:])
        nc.gpsimd.collective_compute(
            "AllGather",
            mybir.AluOpType.bypass,
            replica_groups=[[0, 1, 2, 3, 4, 5, 6, 7]],
            ins=[wag_in[:]],
            outs=[wag_out[:]],
        )
        for g in range(8):
            nc.sync.dma_start(out=w_all[:, 6 * g:6 * (g + 1)], in_=wag_out[g])

        # ---------------- phase 1: tables + projections (own half) ----------
        with tc.tile_pool(name="xn", bufs=2) as xnpool, \
             tc.tile_pool(name="xT", bufs=2) as xTpool, \
             tc.tile_pool(name="rp", bufs=2) as rpool, \
             tc.tile_pool(name="tps", bufs=2, space="PSUM") as tppool, \
             tc.tile_pool(name="tbl", bufs=1, space="PSUM") as tblpool, \
             tc.tile_pool(name="kps", bufs=2, space="PSUM") as kpool, \
             tc.tile_pool(name="vqs", bufs=1, space="PSUM") as vqpool:

            # f32 natural tables from raw f16 halves: [-sin|sin], [cos|cos]
            nc.vector.tensor_scalar_mul(out=sinnF[:, :, 0:64], in0=sinr_t[:],
                                        scalar1=-1.0)
            nc.scalar.copy(out=sinnF[:, :, 64:128], in_=sinr_t[:])
            nc.scalar.copy(out=cosnF[:, :, 0:64], in_=cosr_t[:])
            nc.scalar.copy(out=cosnF[:, :, 64:128], in_=cosr_t[:])
            # K^T-orientation tables by PE-transposing natural blocks
            for jg in range(NBLK):
                tp1 = tblpool.tile([128, 128], F32, tag="tpf")
                nc.tensor.transpose(tp1[:], sinnF[:, jg], idf32_t[:])
                nc.scalar.copy(out=sinKT[:, jg * 128:(jg + 1) * 128], in_=tp1[:])
                tp2 = tblpool.tile([128, 128], F32, tag="tpf")
                nc.tensor.transpose(tp2[:], cosnF[:, jg], idf32_t[:])
                nc.scalar.copy(out=cosKT[:, jg * 128:(jg + 1) * 128], in_=tp2[:])

            def rope_kt(ps, cols):
                """K^T-orientation rope: partition-structured tables."""
                swp = rpool.tile([128, 512], F32, tag="swp")
                m1 = rpool.tile([128, 512], F32, tag="m1")
                nc.scalar.copy(out=swp[0:64, :], in_=ps[64:128, :])
                nc.scalar.copy(out=swp[64:128, :], in_=ps[0:64, :])
                nc.vector.tensor_mul(out=m1[:], in0=ps[:], in1=cosKT[:, cols])
                nc.vector.tensor_mul(out=swp[:], in0=swp[:], in1=sinKT[:, cols])
                nc.vector.tensor_add(out=kt_own[:, cols], in0=m1[:], in1=swp[:])

            def rope_nat(ps, jg):
                """natural-orientation rope: free-structured tables."""
                swp = rpool.tile([128, 128], F32, tag="swn")
                m1 = rpool.tile([128, 128], F32, tag="mn")
                nc.scalar.copy(out=swp[:, 0:64], in_=ps[:, 64:128])
                nc.scalar.copy(out=swp[:, 64:128], in_=ps[:, 0:64])
                nc.vector.tensor_mul(out=m1[:], in0=ps[:], in1=cosnF[:, jg])
                nc.vector.tensor_mul(out=swp[:], in0=swp[:], in1=sinnF[:, jg])
                nc.vector.tensor_add(out=qn_own[:, jg * 128:(jg + 1) * 128],
                                     in0=m1[:], in1=swp[:])

            for rc in range(NBLK // 4):
                xT = xTpool.tile([128, NE, 512], BF16, tag="xT")
                for j in range(4):
                    xn = xnpool.tile([128, EMB], BF16, tag="xn")
                    nc.sync.dma_start(out=xn[:], in_=xh[4 * rc + j])
                    for e in range(NE):
                        tp = tppool.tile([128, 128], BF16, tag="tp")
                        nc.tensor.transpose(tp[:], xn[:, e * 128:(e + 1) * 128],
                                            idbf_t[:])
                        nc.scalar.copy(out=xT[:, e, j * 128:(j + 1) * 128],
                                       in_=tp[:])
                # K^T projection + rope (512 cols at once)
                kps = kpool.tile([128, 512], F32, tag="kps")
                for e in range(NE):
                    nc.tensor.matmul(kps[:], lhsT=wkb(e), rhs=xT[:, e],
                                     start=(e == 0), stop=(e == NE - 1))
                rope_kt(kps, slice(rc * 512, (rc + 1) * 512))
                # V and Q natural per 128-row block
                for j in range(4):
                    jg = 4 * rc + j
                    bsl = slice(j * 128, (j + 1) * 128)
                    vps = vqpool.tile([128, 128], F32, tag="vps")
                    for e in range(NE):
                        nc.tensor.matmul(vps[:], lhsT=xT[:, e, bsl],
                                         rhs=wvb(e),
                                         start=(e == 0), stop=(e == NE - 1))
                    nc.scalar.copy(out=vn_own[:, jg * 128:(jg + 1) * 128],
                                   in_=vps[:])
                    qps = vqpool.tile([128, 128], F32, tag="qps")
                    for e in range(NE):
                        nc.tensor.matmul(qps[:], lhsT=xT[:, e, bsl],
                                         rhs=wqb(e),
                                         start=(e == 0), stop=(e == NE - 1))
                    rope_nat(qps, jg)

        # ---------------- phase 2: pairwise exchange ------------------------
        nc.sync.dma_start(out=ex_in[:, 0:HROWS], in_=qn_own[:])
        nc.sync.dma_start(out=ex_in[:, HROWS:2 * HROWS], in_=kt_own[:])
        nc.sync.dma_start(out=ex_in[:, 2 * HROWS:3 * HROWS], in_=vn_own[:])
        nc.gpsimd.collective_compute(
            "AllGather",
            mybir.AluOpType.bypass,
            replica_groups=[[0, 1], [2, 3], [4, 5], [6, 7]],
            ins=[ex_in[:]],
            outs=[ex_out[:]],
        )
        for g in range(2):
            hb = slice(g * NBLK, (g + 1) * NBLK)
            nc.sync.dma_start(out=qn_full[:, hb], in_=ex_out[g, :, 0:HROWS])
            nc.sync.dma_start(out=kt_full[:, hb],
                              in_=ex_out[g, :, HROWS:2 * HROWS])
            nc.sync.dma_start(out=v_full[:, hb],
                              in_=ex_out[g, :, 2 * HROWS:3 * HROWS])

        # ---------------- phase 3: gather interleaved Q^T -------------------
        with tc.tile_pool(name="gps", bufs=2, space="PSUM") as gpool:
            for J in range(NB):
                gps = gpool.tile([128, 64], F32, tag="g")
                nc.tensor.matmul(gps[:], lhsT=qn_full[:, J], rhs=sel_t[:],
                                 start=True, stop=True)
                nc.scalar.copy(out=qt[:, J * 64:(J + 1) * 64], in_=gps[:])

        # ---------------- phase 4: attention --------------------------------
        with tc.tile_pool(name="pt", bufs=4) as ptpool, \
             tc.tile_pool(name="fin", bufs=2) as finpool, \
             tc.tile_pool(name="stps", bufs=2, space="PSUM") as stpool, \
             tc.tile_pool(name="pvps", bufs=1, space="PSUM") as pvpool, \
             tc.tile_pool(name="sps", bufs=1, space="PSUM") as spool, \
             tc.tile_pool(name="tpps", bufs=1, space="PSUM") as tppool2:

            for v in range(1, C + 1):
                qsl = qt[:, (v - 1) * 256: v * 256]
                kc = 4 * v
                pv_ps = pvpool.tile([128, 256], F32, tag="pv")
                sa_ps = spool.tile([128, 1], F32, tag="sa")
                sb_ps = spool.tile([128, 1], F32, tag="sb")
                for bb in range(kc):
                    st = stpool.tile([128, 256], F32, tag="st")
                    nc.tensor.matmul(st[:], lhsT=kt_full[:, bb], rhs=qsl,
                                     start=True, stop=True)
                    pt = ptpool.tile([128, 256], BF16, tag="pt")
                    nc.scalar.activation(pt[:], st[:],
                                         mybir.ActivationFunctionType.Exp,
                                         scale=scale)
                    d = bb - 4 * (v - 1)
                    if d >= 0:
                        if d > 0:
                            nc.gpsimd.memset(pt[:, 0:64 * d], 0.0)
                        nc.vector.tensor_mul(out=pt[:, 64 * d:64 * d + 64],
                                             in0=pt[:, 64 * d:64 * d + 64],
                                             in1=tri_t[:])
                    nc.tensor.matmul(sa_ps[:], lhsT=pt[:, 0:128], rhs=ones_t[:],
                                     start=(bb == 0), stop=(bb == kc - 1))
                    nc.tensor.matmul(sb_ps[:], lhsT=pt[:, 128:256], rhs=ones_t[:],
                                     start=(bb == 0), stop=(bb == kc - 1))
                    nc.tensor.matmul(pv_ps[:], lhsT=v_full[:, bb], rhs=pt[:],
                                     start=(bb == 0), stop=(bb == kc - 1))

                # finalize: transpose out^T back to natural, divide by sums
                outt = finpool.tile([128, 256], F32, tag="outt")
                nc.scalar.copy(out=outt[:], in_=pv_ps[:])
                srec = finpool.tile([128, 2], F32, tag="srec")
                nc.vector.reciprocal(out=srec[:, 0:1], in_=sa_ps[:])
                nc.vector.reciprocal(out=srec[:, 1:2], in_=sb_ps[:])
                for half in range(2):
                    tp = tppool2.tile([128, 128], F32, tag="tp")
                    nc.tensor.transpose(tp[:], outt[:, half * 128:(half + 1) * 128],
                                        idf32_t[:])
                    ot = finpool.tile([128, 128], BF16, tag="ot")
                    nc.vector.tensor_scalar_mul(out=ot[:], in0=tp[:],
                                                scalar1=srec[:, half:half + 1])
                    r0 = (v - 1) * 256 + half * 128
                    nc.sync.dma_start(out=out[r0:r0 + 128, :], in_=ot[:])

        const_cm.__exit__(None, None, None)

    nc.finalize()
    return nc


# ---------------- host-side prep ----------------

def _bf16_trunc_view(a_f32):
    """f32 ndarray -> bf16 truncation as a zero-copy strided view."""
    v = a_f32.view(np.uint16)[..., 1::2]
    return v.view(ml_dtypes.bfloat16)


def _bf16_to_f32(a_bf16):
    """fast widening cast (ml_dtypes' own astype is slow on this host)."""
    u = np.asarray(a_bf16).view(np.uint16).astype(np.uint32) << 16
    return u.view(np.float32)


def _perm_cols(w):
    """Interleaved rope pairs -> half-split: [:,0:64]=even cols, [:,64:]=odd."""
    return np.concatenate([w[:, 0::2], w[:, 1::2]], axis=1)


def _wfmt(w):
    return np.ascontiguousarray(
        w.astype(ml_dtypes.bfloat16).reshape(NE, 128, 128).transpose(1, 0, 2))


def _rawtbl(t_half):
    """[2048, 64] raw table slice -> [128, 16, 64] partition-first f16."""
    return np.ascontiguousarray(
        t_half.astype(np.float16).reshape(NBLK, 128, 64).transpose(1, 0, 2))


def make_in_maps(x, sin, cos, W_Q, W_K, W_V):
    xb = _bf16_trunc_view(np.ascontiguousarray(x) if not x.flags.c_contiguous else x)

    wstack = np.concatenate([_wfmt(_perm_cols(W_Q)), _wfmt(_perm_cols(W_K)),
                             _wfmt(W_V)], axis=1)   # [128, 48, 128] bf16

    tbl = {}
    for h in range(2):
        rows = slice(HROWS * h, HROWS * (h + 1))
        tbl[h] = (_rawtbl(sin[rows]), _rawtbl(cos[rows]))

    eye = np.eye(128, dtype=ml_dtypes.bfloat16)
    sel = {h: np.ascontiguousarray(eye[:, 64 * h:64 * h + 64]) for h in range(2)}
    kk = np.arange(128)[:, None]
    qq = np.arange(64)[None, :]
    tri = {0: (kk <= qq).astype(ml_dtypes.bfloat16),
           1: (kk <= 64 + qq).astype(ml_dtypes.bfloat16)}

    in_maps = []
    for c in range(2 * BSZ):
        b, h = c // 2, c % 2
        in_maps.append({
            "xh": xb[b, HROWS * h:HROWS * (h + 1)].reshape(NBLK, 128, EMB),
            "wsh": wstack[:, 6 * c:6 * (c + 1)],
            "sinr": tbl[h][0], "cosr": tbl[h][1],
            "sel": sel[h], "tri": tri[h],
        })
    return in_maps


_NC_CACHE = {}


def run(x, sin, cos, W_Q, W_K, W_V, trace=False):
    from concourse.bass_utils import run_bass_kernel_spmd
    if "nc" not in _NC_CACHE:
        _NC_CACHE["nc"] = build_nc()
    nc = _NC_CACHE["nc"]
    in_maps = make_in_maps(x, sin, cos, W_Q, W_K, W_V)
    res = run_bass_kernel_spmd(nc, in_maps, list(range(2 * BSZ)), trace=trace)
    out_full = np.empty((BSZ, SEQ, 128), dtype=np.float32)
    ov = out_full.reshape(BSZ, NB, 2, 64, 128)
    for c in range(2 * BSZ):
        b, h = c // 2, c % 2
        o = _bf16_to_f32(res.results[c]["out"]).reshape(NB, 64, 128)
        ov[b, :, h] = o
    return out_full, res


def kernel(x, mask, sin, cos, W_Q, W_V, W_K):
    out, _ = run(np.asarray(x), np.asarray(sin), np.asarray(cos),
                 np.asarray(W_Q), np.asarray(W_K), np.asarray(W_V))
    return out


# revision 18
# speedup vs baseline: 4.7886x; 1.1249x over previous
"""Trainium2 Bass kernel: single-head causal attention with RoPE.

Reference computation (per batch b of 4):
  Q = rope(x @ W_Q), K = rope(x @ W_K), V = x @ W_V      x: [4096, 2048], W: [2048, 128]
  out = softmax(mask(Q K^T / sqrt(128))) @ V             out: [4096, 128]

The wall-clock cost in this environment is dominated by the host->device
tunnel (~65 MB/s) and single-CPU host packing, not device compute. So this
version ships the *minimum* bytes with *zero* host-side packing passes:

- x is shipped as bf16 obtained by a strided uint16 truncation VIEW of the
  f32 input (no cast pass; the one copy happens inside run_bass_kernel_spmd's
  np.concatenate). Each core receives only its own contiguous half-batch
  (8 MB), the true lower bound for 8 cores.
- Each core computes Q/K (roped) and V for its own 2048 contiguous rows,
  then the two cores of a batch exchange them with a pairwise AllGather
  (on-device DRAM bounce, ~3 MB) so both see the full-batch K/V/Q.
- Query ownership for the attention phase is interleaved (core h owns rows
  128J + 64h + r) which makes causal work and the instruction stream
  identical across cores; the per-core interleaved Q columns are gathered
  on-device with a selection-matrix matmul (sel is per-core DATA).
- All transposes (x -> x^T for the projections, tables, final output) are
  PE transposes on device; rope tables are built on device from small
  per-core bf16 slices; output is normalized on device and shipped back
  as bf16 [2048, 128] per core.
"""

import math
import sys

sys.path.insert(0, "/opt/trn_rl_repo")

import numpy as np
import ml_dtypes

import concourse.bass as bass
import concourse.mybir as mybir
import concourse.tile as tile
from concourse import bacc

BF16 = mybir.dt.bfloat16
F16 = mybir.dt.float16
F32 = mybir.dt.float32

SEQ, EMB, BSZ, DH = 4096, 2048, 4, 128
HROWS = SEQ // 2          # rows owned per core (contiguous half)
NBLK = HROWS // 128       # 16 own 128-row blocks
NE = EMB // 128           # 16 emb chunks
NB = SEQ // 128           # 32 kv blocks
C = NB // 4               # 8 attention chunks of 256 packed q rows


def build_nc():
    scale = 1.0 / math.sqrt(float(DH))
    nc = bacc.Bacc("TRN2", num_devices=8)

    xh = nc.declare_dram_parameter("xh", [NBLK, 128, EMB], BF16, isOutput=False)
    # all small inputs packed into one bf16-container blob (fewer PJRT arrays
    # -> much faster axon transfer). Rows of 2048 els:
    #   0:48    wsh   [128,6,128] bf16 — weight shard (stacked [wq|wk|wv], 6/core)
    #   48:112  sinr  [128,16,64] f16 bits
    #   112:176 cosr  [128,16,64] f16 bits
    #   176:180 sel   [128,64] bf16
    #   180:184 tri   [128,64] bf16
    blob = nc.declare_dram_parameter("blob", [184, 2048], BF16, isOutput=False)
    out = nc.declare_dram_parameter("out", [HROWS, 128], BF16, isOutput=True)

    ident_bf = nc.inline_tensor(np.eye(128, dtype=ml_dtypes.bfloat16), name="idbf")
    ident_f32 = nc.inline_tensor(np.eye(128, dtype=np.float32), name="idf32")

    # pairwise exchange buffers: sections [q_nat | k_T | v_nat], each [128, 2048]
    ex_in = nc.dram_tensor("ex_in", [128, 3 * HROWS], BF16)
    ex_out = nc.dram_tensor("ex_out", [2, 128, 3 * HROWS], BF16)
    # weight-reassembly AllGather (collectives cannot read IO tensors directly)
    wag_in = nc.dram_tensor("wag_in", [128, 6, 128], BF16)
    wag_out = nc.dram_tensor("wag_out", [8, 128, 6, 128], BF16)

    with tile.TileContext(nc) as tc:
        const_cm = tc.tile_pool(name="const", bufs=1)
        cp = const_cm.__enter__()

        w_all = cp.tile([128, 48, 128], BF16, tag="w_all")
        wqb = lambda e: w_all[:, e]
        wkb = lambda e: w_all[:, NE + e]
        wvb = lambda e: w_all[:, 2 * NE + e]
        sinr_t = cp.tile([128, NBLK, 64], F16, tag="sinr")
        cosr_t = cp.tile([128, NBLK, 64], F16, tag="cosr")
        sel_t = cp.tile([128, 64], BF16, tag="sel")
        tri_t = cp.tile([128, 64], BF16, tag="tri")
        idbf_t = cp.tile([128, 128], BF16, tag="idbf")
        idf32_t = cp.tile([128, 128], F32, tag="idf32")
        ones_t = cp.tile([128, 1], BF16, tag="ones")

        sinnF = cp.tile([128, NBLK, 128], F32, tag="sinnF")   # natural f32
        cosnF = cp.tile([128, NBLK, 128], F32, tag="cosnF")
        sinKT = cp.tile([128, HROWS], F32, tag="sinKT")       # K^T orientation f32
        cosKT = cp.tile([128, HROWS], F32, tag="cosKT")

        kt_own = cp.tile([128, HROWS], BF16, tag="kt_own")    # roped K^T, own half
        qn_own = cp.tile([128, HROWS], BF16, tag="qn_own")    # roped Q natural, own half
        vn_own = cp.tile([128, HROWS], BF16, tag="vn_own")    # V natural, own half

        kt_full = cp.tile([128, NB, 128], BF16, tag="kt_full")
        qn_full = cp.tile([128, NB, 128], BF16, tag="qn_full")
        v_full = cp.tile([128, NB, 128], BF16, tag="v_full")
        qt = cp.tile([128, HROWS], BF16, tag="qt")            # gathered Q^T, packed

        nc.sync.dma_start(out=sinr_t[:], in_=blob[48:112].bitcast(F16))
        nc.sync.dma_start(out=cosr_t[:], in_=blob[112:176].bitcast(F16))
        nc.sync.dma_start(out=sel_t[:], in_=blob[176:180])
        nc.sync.dma_start(out=tri_t[:], in_=blob[180:184])
        nc.sync.dma_start(out=idbf_t[:], in_=ident_bf[:])
        nc.sync.dma_start(out=idf32_t[:], in_=ident_f32[:])
        nc.gpsimd.memset(ones_t[:], 1.0)

        # reassemble full weights from the 8 per-core shards
        wsh_t = cp.tile([128, 6, 128], BF16, tag="wsh")
        nc.sync.dma_start(out=wsh_t[:], in_=blob[0:48])
        nc.sync.dma_start(out=wag_in[:], in_=wsh_t[:])
        nc.gpsimd.collective_compute(
            "AllGather",
            mybir.AluOpType.bypass,
            replica_groups=[[0, 1, 2, 3, 4, 5, 6, 7]],
            ins=[wag_in[:]],
            outs=[wag_out[:]],
        )
        for g in range(8):
            nc.sync.dma_start(out=w_all[:, 6 * g:6 * (g + 1)], in_=wag_out[g])

        # ---------------- phase 1: tables + projections (own half) ----------
        with tc.tile_pool(name="xn", bufs=2) as xnpool, \
             tc.tile_pool(name="xT", bufs=2) as xTpool, \
             tc.tile_pool(name="rp", bufs=2) as rpool, \
             tc.tile_pool(name="tps", bufs=2, space="PSUM") as tppool, \
             tc.tile_pool(name="tbl", bufs=1, space="PSUM") as tblpool, \
             tc.tile_pool(name="kps", bufs=2, space="PSUM") as kpool, \
             tc.tile_pool(name="vqs", bufs=1, space="PSUM") as vqpool:

            # f32 natural tables from raw f16 halves: [-sin|sin], [cos|cos]
            nc.vector.tensor_scalar_mul(out=sinnF[:, :, 0:64], in0=sinr_t[:],
                                        scalar1=-1.0)
            nc.scalar.copy(out=sinnF[:, :, 64:128], in_=sinr_t[:])
            nc.scalar.copy(out=cosnF[:, :, 0:64], in_=cosr_t[:])
            nc.scalar.copy(out=cosnF[:, :, 64:128], in_=cosr_t[:])
            # K^T-orientation tables by PE-transposing natural blocks
            for jg in range(NBLK):
                tp1 = tblpool.tile([128, 128], F32, tag="tpf")
                nc.tensor.transpose(tp1[:], sinnF[:, jg], idf32_t[:])
                nc.scalar.copy(out=sinKT[:, jg * 128:(jg + 1) * 128], in_=tp1[:])
                tp2 = tblpool.tile([128, 128], F32, tag="tpf")
                nc.tensor.transpose(tp2[:], cosnF[:, jg], idf32_t[:])
                nc.scalar.copy(out=cosKT[:, jg * 128:(jg + 1) * 128], in_=tp2[:])

            def rope_kt(ps, cols):
                """K^T-orientation rope: partition-structured tables."""
                swp = rpool.tile([128, 512], F32, tag="swp")
                m1 = rpool.tile([128, 512], F32, tag="m1")
                nc.scalar.copy(out=swp[0:64, :], in_=ps[64:128, :])
                nc.scalar.copy(out=swp[64:128, :], in_=ps[0:64, :])
                nc.vector.tensor_mul(out=m1[:], in0=ps[:], in1=cosKT[:, cols])
                nc.vector.tensor_mul(out=swp[:], in0=swp[:], in1=sinKT[:, cols])
                nc.vector.tensor_add(out=kt_own[:, cols], in0=m1[:], in1=swp[:])

            def rope_nat(ps, jg):
                """natural-orientation rope: free-structured tables."""
                swp = rpool.tile([128, 128], F32, tag="swn")
                m1 = rpool.tile([128, 128], F32, tag="mn")
                nc.scalar.copy(out=swp[:, 0:64], in_=ps[:, 64:128])
                nc.scalar.copy(out=swp[:, 64:128], in_=ps[:, 0:64])
                nc.vector.tensor_mul(out=m1[:], in0=ps[:], in1=cosnF[:, jg])
                nc.vector.tensor_mul(out=swp[:], in0=swp[:], in1=sinnF[:, jg])
                nc.vector.tensor_add(out=qn_own[:, jg * 128:(jg + 1) * 128],
                                     in0=m1[:], in1=swp[:])

            for rc in range(NBLK // 4):
                xT = xTpool.tile([128, NE, 512], BF16, tag="xT")
                for j in range(4):
                    xn = xnpool.tile([128, EMB], BF16, tag="xn")
                    nc.sync.dma_start(out=xn[:], in_=xh[4 * rc + j])
                    for e in range(NE):
                        tp = tppool.tile([128, 128], BF16, tag="tp")
                        nc.tensor.transpose(tp[:], xn[:, e * 128:(e + 1) * 128],
                                            idbf_t[:])
                        nc.scalar.copy(out=xT[:, e, j * 128:(j + 1) * 128],
                                       in_=tp[:])
                # K^T projection + rope (512 cols at once)
                kps = kpool.tile([128, 512], F32, tag="kps")
                for e in range(NE):
                    nc.tensor.matmul(kps[:], lhsT=wkb(e), rhs=xT[:, e],
                                     start=(e == 0), stop=(e == NE - 1))
                rope_kt(kps, slice(rc * 512, (rc + 1) * 512))
                # V and Q natural per 128-row block
                for j in range(4):
                    jg = 4 * rc + j
                    bsl = slice(j * 128, (j + 1) * 128)
                    vps = vqpool.tile([128, 128], F32, tag="vps")
                    for e in range(NE):
                        nc.tensor.matmul(vps[:], lhsT=xT[:, e, bsl],
                                         rhs=wvb(e),
                                         start=(e == 0), stop=(e == NE - 1))
                    nc.scalar.copy(out=vn_own[:, jg * 128:(jg + 1) * 128],
                                   in_=vps[:])
                    qps = vqpool.tile([128, 128], F32, tag="qps")
                    for e in range(NE):
                        nc.tensor.matmul(qps[:], lhsT=xT[:, e, bsl],
                                         rhs=wqb(e),
                                         start=(e == 0), stop=(e == NE - 1))
                    rope_nat(qps, jg)

        # ---------------- phase 2: pairwise exchange ------------------------
        nc.sync.dma_start(out=ex_in[:, 0:HROWS], in_=qn_own[:])
        nc.sync.dma_start(out=ex_in[:, HROWS:2 * HROWS], in_=kt_own[:])
        nc.sync.dma_start(out=ex_in[:, 2 * HROWS:3 * HROWS], in_=vn_own[:])
        nc.gpsimd.collective_compute(
            "AllGather",
            mybir.AluOpType.bypass,
            replica_groups=[[0, 1], [2, 3], [4, 5], [6, 7]],
            ins=[ex_in[:]],
            outs=[ex_out[:]],
        )
        for g in range(2):
            hb = slice(g * NBLK, (g + 1) * NBLK)
            nc.sync.dma_start(out=qn_full[:, hb], in_=ex_out[g, :, 0:HROWS])
            nc.sync.dma_start(out=kt_full[:, hb],
                              in_=ex_out[g, :, HROWS:2 * HROWS])
            nc.sync.dma_start(out=v_full[:, hb],
                              in_=ex_out[g, :, 2 * HROWS:3 * HROWS])

        # ---------------- phase 3: gather interleaved Q^T -------------------
        with tc.tile_pool(name="gps", bufs=2, space="PSUM") as gpool:
            for J in range(NB):
                gps = gpool.tile([128, 64], F32, tag="g")
                nc.tensor.matmul(gps[:], lhsT=qn_full[:, J], rhs=sel_t[:],
                                 start=True, stop=True)
                nc.scalar.copy(out=qt[:, J * 64:(J + 1) * 64], in_=gps[:])

        # ---------------- phase 4: attention --------------------------------
        with tc.tile_pool(name="pt", bufs=4) as ptpool, \
             tc.tile_pool(name="fin", bufs=2) as finpool, \
             tc.tile_pool(name="stps", bufs=2, space="PSUM") as stpool, \
             tc.tile_pool(name="pvps", bufs=1, space="PSUM") as pvpool, \
             tc.tile_pool(name="sps", bufs=1, space="PSUM") as spool, \
             tc.tile_pool(name="tpps", bufs=1, space="PSUM") as tppool2:

            for v in range(1, C + 1):
                qsl = qt[:, (v - 1) * 256: v * 256]
                kc = 4 * v
                pv_ps = pvpool.tile([128, 256], F32, tag="pv")
                sa_ps = spool.tile([128, 1], F32, tag="sa")
                sb_ps = spool.tile([128, 1], F32, tag="sb")
                for bb in range(kc):
                    st = stpool.tile([128, 256], F32, tag="st")
                    nc.tensor.matmul(st[:], lhsT=kt_full[:, bb], rhs=qsl,
                                     start=True, stop=True)
                    pt = ptpool.tile([128, 256], BF16, tag="pt")
                    nc.scalar.activation(pt[:], st[:],
                                         mybir.ActivationFunctionType.Exp,
                                         scale=scale)
                    d = bb - 4 * (v - 1)
                    if d >= 0:
                        if d > 0:
                            nc.gpsimd.memset(pt[:, 0:64 * d], 0.0)
                        nc.vector.tensor_mul(out=pt[:, 64 * d:64 * d + 64],
                                             in0=pt[:, 64 * d:64 * d + 64],
                                             in1=tri_t[:])
                    nc.tensor.matmul(sa_ps[:], lhsT=pt[:, 0:128], rhs=ones_t[:],
                                     start=(bb == 0), stop=(bb == kc - 1))
                    nc.tensor.matmul(sb_ps[:], lhsT=pt[:, 128:256], rhs=ones_t[:],
                                     start=(bb == 0), stop=(bb == kc - 1))
                    nc.tensor.matmul(pv_ps[:], lhsT=v_full[:, bb], rhs=pt[:],
                                     start=(bb == 0), stop=(bb == kc - 1))

                # finalize: transpose out^T back to natural, divide by sums
                outt = finpool.tile([128, 256], F32, tag="outt")
                nc.scalar.copy(out=outt[:], in_=pv_ps[:])
                srec = finpool.tile([128, 2], F32, tag="srec")
                nc.vector.reciprocal(out=srec[:, 0:1], in_=sa_ps[:])
                nc.vector.reciprocal(out=srec[:, 1:2], in_=sb_ps[:])
                for half in range(2):
                    tp = tppool2.tile([128, 128], F32, tag="tp")
                    nc.tensor.transpose(tp[:], outt[:, half * 128:(half + 1) * 128],
                                        idf32_t[:])
                    ot = finpool.tile([128, 128], BF16, tag="ot")
                    nc.vector.tensor_scalar_mul(out=ot[:], in0=tp[:],
                                                scalar1=srec[:, half:half + 1])
                    r0 = (v - 1) * 256 + half * 128
                    nc.sync.dma_start(out=out[r0:r0 + 128, :], in_=ot[:])

        const_cm.__exit__(None, None, None)

    nc.finalize()
    return nc


# ---------------- host-side prep ----------------

def _bf16_trunc_view(a_f32):
    """f32 ndarray -> bf16 truncation as a zero-copy strided view."""
    v = a_f32.view(np.uint16)[..., 1::2]
    return v.view(ml_dtypes.bfloat16)


def _bf16_to_f32(a_bf16):
    """fast widening cast (ml_dtypes' own astype is slow on this host)."""
    u = np.asarray(a_bf16).view(np.uint16).astype(np.uint32) << 16
    return u.view(np.float32)


def _perm_cols(w):
    """Interleaved rope pairs -> half-split: [:,0:64]=even cols, [:,64:]=odd."""
    return np.concatenate([w[:, 0::2], w[:, 1::2]], axis=1)


def _wfmt(w):
    return np.ascontiguousarray(
        w.astype(ml_dtypes.bfloat16).reshape(NE, 128, 128).transpose(1, 0, 2))


def _rawtbl(t_half):
    """[2048, 64] raw table slice -> [128, 16, 64] partition-first f16."""
    return np.ascontiguousarray(
        t_half.astype(np.float16).reshape(NBLK, 128, 64).transpose(1, 0, 2))


def make_in_maps(x, sin, cos, W_Q, W_K, W_V):
    xb = _bf16_trunc_view(np.ascontiguousarray(x) if not x.flags.c_contiguous else x)

    wstack = np.concatenate([_wfmt(_perm_cols(W_Q)), _wfmt(_perm_cols(W_K)),
                             _wfmt(W_V)], axis=1)   # [128, 48, 128] bf16

    tbl = {}
    for h in range(2):
        rows = slice(HROWS * h, HROWS * (h + 1))
        tbl[h] = (_rawtbl(sin[rows]), _rawtbl(cos[rows]))

    eye = np.eye(128, dtype=ml_dtypes.bfloat16)
    sel = {h: np.ascontiguousarray(eye[:, 64 * h:64 * h + 64]) for h in range(2)}
    kk = np.arange(128)[:, None]
    qq = np.arange(64)[None, :]
    tri = {0: (kk <= qq).astype(ml_dtypes.bfloat16),
           1: (kk <= 64 + qq).astype(ml_dtypes.bfloat16)}

    in_maps = []
    for c in range(2 * BSZ):
        b, h = c // 2, c % 2
        blob = np.empty((184, 2048), dtype=np.uint16)
        bv = blob.reshape(-1)
        bv[0:98304] = wstack[:, 6 * c:6 * (c + 1)].view(np.uint16).reshape(-1)
        bv[98304:229376] = tbl[h][0].view(np.uint16).reshape(-1)
        bv[229376:360448] = tbl[h][1].view(np.uint16).reshape(-1)
        bv[360448:368640] = sel[h].view(np.uint16).reshape(-1)
        bv[368640:376832] = tri[h].view(np.uint16).reshape(-1)
        in_maps.append({
            "xh": xb[b, HROWS * h:HROWS * (h + 1)].reshape(NBLK, 128, EMB),
            "blob": blob.view(ml_dtypes.bfloat16),
        })
    return in_maps


_NC_CACHE = {}


def run(x, sin, cos, W_Q, W_K, W_V, trace=False):
    from concourse.bass_utils import run_bass_kernel_spmd
    if "nc" not in _NC_CACHE:
        _NC_CACHE["nc"] = build_nc()
    nc = _NC_CACHE["nc"]
    in_maps = make_in_maps(x, sin, cos, W_Q, W_K, W_V)
    res = run_bass_kernel_spmd(nc, in_maps, list(range(2 * BSZ)), trace=trace)
    out_full = np.empty((BSZ, SEQ, 128), dtype=np.float32)
    ov = out_full.reshape(BSZ, NB, 2, 64, 128)
    for c in range(2 * BSZ):
        b, h = c // 2, c % 2
        o = _bf16_to_f32(res.results[c]["out"]).reshape(NB, 64, 128)
        ov[b, :, h] = o
    return out_full, res


def kernel(x, mask, sin, cos, W_Q, W_V, W_K):
    out, _ = run(np.asarray(x), np.asarray(sin), np.asarray(cos),
                 np.asarray(W_Q), np.asarray(W_K), np.asarray(W_V))
    return out
